# revision 27
# baseline (speedup 1.0000x reference)
"""Distributed LNO block kernel for 8 TRN2 NeuronCores.

Math (reference):
    phi   = x @ phi_w + phi_b                      [B,N,r]
    psi   = (x @ psi_w + psi_b).reshape(B,N,r,C)   [B,N,r,C]  (never materialized)
    integral_c = einsum('bnrc,bnc->brc', psi, x)/N [B,r,C]
    out   = gelu(x @ W_w + W_b + phi @ integral_c)

Key identity: integral_c[b,r,c] = sum_k psi_w[k, r*C+c] * G[b,k,c] + psi_b[r*C+c] * m[b,c]
with G = (x^T x)/N (per-batch Gram, [B,C,C]) and m = mean_n x.  This removes the
34 GFLOP psi matmul entirely.

Distribution: shard N by 8 (each core: 1024 pts of both batches).
  1. per-core scaled Gram G_loc/N (+mean col via a baked-in ones column) on PE,
     AllReduce (fp16, 66KB) -> global G/N
  2. per-core r-slice (8 ranks) of the integral via one DVE broadcast-mul and
     one DVE free-axis reduce (uses G's symmetry: G^T = G, so no G transpose),
     AllGather (fp16, 4KB/core) -> full integral everywhere
  3. per-core: w_x and phi @ integral accumulate into the SAME PSUM bank (one
     accumulation group per 2KB zero-region), gelu straight out of PSUM,
     batched 256KB stores.

Everything is fp16 on the wire and in the matmuls (fp32 accumulation); inputs
are pre-cast and pre-transposed on the host so every DMA is contiguous and no
x/psi_w transposes run on the device.  rel-err vs the fp32 reference ~5e-4.
"""

import sys

sys.path.insert(0, "/opt/trn_rl_repo")

import numpy as np

import concourse.bass as bass
import concourse.bacc as bacc
import concourse.mybir as mybir
import concourse.tile as tile
from concourse.tile import add_dep_helper
from concourse.bass_utils import run_bass_kernel_spmd

FP = mybir.dt.float32
HF = mybir.dt.float16
AF = mybir.ActivationFunctionType
ALU = mybir.AluOpType

B, N, C, R = 2, 8192, 128, 64
NCORES = 8
NSH = N // NCORES      # 1024 points per core
RL = R // NCORES       # 8 rank rows per core
NT = NSH // 128        # 8 n-tiles of 128 per batch
CP1 = C + 1            # G columns + mean column

_CACHE = {}
LAST_RESULTS = None


def _build():
    nc = bacc.Bacc(
        "TRN2", target_bir_lowering=False, debug=False, num_devices=NCORES
    )

    xt_in = nc.dram_tensor("xt", [128, B * NT * 129], HF, kind="ExternalInput")
    xT_in = nc.dram_tensor("xT", [128, B * NSH], HF, kind="ExternalInput")
    psiwT_in = nc.dram_tensor("psiwT", [128, RL * C], HF, kind="ExternalInput")
    psibF_in = nc.dram_tensor("psibF", [R, C], HF, kind="ExternalInput")
    phiw_in = nc.dram_tensor("phi_w", [C, R], HF, kind="ExternalInput")
    phibT_in = nc.dram_tensor("phibT", [R, 1], FP, kind="ExternalInput")
    ww_in = nc.dram_tensor("W_w", [C, C], HF, kind="ExternalInput")
    wb_in = nc.dram_tensor("W_b", [1, C], HF, kind="ExternalInput")
    id_in = nc.dram_tensor("ident", [128, 128], HF, kind="ExternalInput")
    out_ext = nc.dram_tensor("out", [B, NSH, C], FP, kind="ExternalOutput")

    with tile.TileContext(nc) as tc:
        with (
            tc.tile_pool(name="big", bufs=1) as bigp,
            tc.tile_pool(name="outs", bufs=4) as outp,
            tc.tile_pool(name="dram", bufs=1, space="DRAM") as dramp,
            tc.tile_pool(name="gmps", bufs=1, space="PSUM") as gmp,
            tc.tile_pool(name="wrk", bufs=2, space="PSUM") as wrkp,
            tc.tile_pool(name="wrkh", bufs=1, space="PSUM") as wrkhp,
            tc.tile_pool(name="kops", bufs=4, space="PSUM") as kop,
        ):
            # ---- static SBUF tiles ----
            id_hf = bigp.tile([128, 128], HF, name="id_hf")
            x_hf = bigp.tile([128, B * NT * 129], HF, name="x_hf")
            xT_hf = bigp.tile([128, B * NSH], HF, name="xT_hf")
            psiwT_hf = bigp.tile([128, RL * C], HF, name="psiwT_hf")
            psibF_hf = bigp.tile([R, C], HF, name="psibF_hf")
            ones8_hf = bigp.tile([RL, R], HF, name="ones8_hf")
            mg_hf = bigp.tile([RL, B * C], HF, name="mg_hf")
            mtmp_f = bigp.tile([R, C], FP, name="mtmp_f")
            phiw_hf = bigp.tile([128, R], HF, name="phiw_hf")
            phibT_f = bigp.tile([R, 1], FP, name="phibT_f")
            ww_hf = bigp.tile([128, C], HF, name="ww_hf")
            phiT_hf = bigp.tile([R + 1, B * NSH], HF, name="phiT_hf")
            gloc_hf = bigp.tile([128, B * C], HF, name="gloc_hf")
            graw_hf = bigp.tile([128, B * C], HF, name="graw_hf")
            prod_hf = bigp.tile([128, B * RL * C], HF, name="prod_hf")
            red_f = bigp.tile([128, B * RL], FP, name="red_f")
            intT_hf = bigp.tile([128, B * RL + B], HF, name="intT_hf")
            intRow_hf = bigp.tile([B * RL + B, C], HF, name="intRow_hf")
            integ_hf = bigp.tile([R + 1, B * C], HF, name="integ_hf")

            # ---- DRAM bounce buffers for collectives ----
            g_bounce = dramp.tile([128, B * C], HF, name="g_bounce")
            g_red = dramp.tile(
                [128, B * C], HF, name="g_red", addr_space="Shared"
            )
            i_bounce = dramp.tile([B * RL + B, C], HF, name="i_bounce")
            i_gath = dramp.tile(
                [NCORES, B * RL + B, C], HF, name="i_gath", addr_space="Shared"
            )

            nc.vector.memset(phiT_hf[R : R + 1, :], 1.0)
            nc.vector.memset(ones8_hf[:], 1.0)
            # tiny dummy gelu: forces the ACT function tables to load now
            # instead of lazily right before the first real gelu in the tail
            warm_f = bigp.tile([1, 8], FP, name="warm_f")
            nc.vector.memset(warm_f[:], 0.0)
            nc.scalar.activation(warm_f[:], warm_f[:], AF.Gelu)

            # ---- x tiles: 6 contiguous 86KB DMAs round-robin on 3 queues ----
            q = B * NT * 129 // 6
            xq = [nc.sync.dma_start, nc.scalar.dma_start, nc.gpsimd.dma_start]
            for i in range(6):
                xq[i % 3](
                    x_hf[:, i * q : (i + 1) * q], xt_in[:, i * q : (i + 1) * q]
                )

            # ---- Gram matmuls (+ mean col via the baked-in ones column) ----
            g_ps = gmp.tile([128, B * CP1], FP, name="g_ps")
            for b in range(B):
                for j in range(NT):
                    t = b * NT + j
                    xt = x_hf[:, t * 129 : t * 129 + 128]
                    xt1 = x_hf[:, t * 129 : t * 129 + 129]
                    nc.tensor.matmul(
                        g_ps[:, b * CP1 : (b + 1) * CP1],
                        xt,
                        xt1,
                        start=(j == 0),
                        stop=(j == NT - 1),
                    )
            # evict pre-scaled by 1/N so the AllReduce carries G/N directly;
            # only the G block goes on the wire (65536B -> Mesh algorithm),
            # the local mean columns ride the AllGather instead.
            gpview = g_ps[:].rearrange("p (b w) -> p b w", w=CP1)
            bdma = [nc.scalar.dma_start, nc.sync.dma_start]
            for b in range(B):
                nc.vector.tensor_scalar_mul(
                    gloc_hf[:, b * C : (b + 1) * C],
                    gpview[:, b, 0:C],
                    1.0 / N,
                )
                bdma[b](
                    g_bounce[:, b * C : (b + 1) * C],
                    gloc_hf[:, b * C : (b + 1) * C],
                )
            nc.vector.tensor_scalar_mul(
                intT_hf[:, B * RL : B * RL + B], gpview[:, :, C], 1.0 / N
            )
            ar_cc = nc.gpsimd.collective_compute(
                "AllReduce",
                ALU.add,
                replica_groups=[list(range(NCORES))],
                ins=[g_bounce.opt()],
                outs=[g_red.opt()],
            )

            # ---- remaining loads: demoted priority so the scheduler keeps
            # them behind the critical pre-trigger chain; all of these are
            # consumed only under/after the collective waits ----
            _late = []
            _late.append(nc.scalar.dma_start(xT_hf[:], xT_in[:]))
            _late.append(nc.gpsimd.dma_start(phiw_hf[:], phiw_in[:]))
            _late.append(nc.gpsimd.dma_start(ww_hf[:], ww_in[:]))
            _late.append(nc.gpsimd.dma_start(id_hf[:], id_in[:]))
            _late.append(nc.gpsimd.dma_start(psiwT_hf[:], psiwT_in[:]))
            _late.append(nc.gpsimd.dma_start(psibF_hf[:], psibF_in[:]))
            _late.append(nc.gpsimd.dma_start(phibT_f[:], phibT_in[:]))
            for b in range(B):
                _late.append(
                    nc.gpsimd.dma_start(
                        integ_hf[R : R + 1, b * C : (b + 1) * C], wb_in[:]
                    )
                )
            for _di in _late:
                add_dep_helper(
                    _di.ins,
                    ar_cc.ins,
                    sync=False,
                    reason="load after the AllReduce trigger",
                )

            # ---- overlaps the AllReduce: phi + the w_x half of the tail ----
            _phi_evicts = []
            for ch in range(4):
                phi_ps = wrkp.tile([R, 512], FP, tag="w", name=f"phi_ps{ch}")
                cols = slice(ch * 512, (ch + 1) * 512)
                nc.tensor.matmul(
                    phi_ps[:], phiw_hf[:], xT_hf[:, cols], start=True, stop=True
                )
                _phi_evicts.append(
                    nc.vector.tensor_scalar_add(
                        phiT_hf[:R, cols], phi_ps[:], phibT_f[:]
                    )
                )

            # w_x matmuls need only xT/W_w — issue them now so they run under
            # the collective waits; phi@integral accumulates on top after the
            # AllGather lands.  One accumulation group per PSUM bank: start
            # only on the bank's first matmul (start=True clears has_written
            # for the entire 2KB zero-region).
            ko_banks = []
            for g in range(4):
                ko_ps = kop.tile([128, 512], FP, tag="ko", name=f"ko{g}")
                ko_banks.append(ko_ps)
                for i in range(4):
                    t = g * 4 + i
                    nc.tensor.matmul(
                        ko_ps[:, i * 128 : (i + 1) * 128],
                        xT_hf[:, t * 128 : (t + 1) * 128],
                        ww_hf[:],
                        start=(i == 0),
                        stop=False,
                    )

            # ---- post-AllReduce: integral r-slice on DVE, pipelined per
            # batch (each batch's 32KB G half loads on its own queue) ----
            gdma = [nc.sync.dma_start, nc.scalar.dma_start]
            for b in range(B):
                gdma[b](
                    graw_hf[:, b * C : (b + 1) * C],
                    g_red[:, b * C : (b + 1) * C],
                )
            gview = graw_hf[:].rearrange("p (b w) -> p b w", w=C)
            pw3 = psiwT_hf[:].rearrange("p (rl k) -> p rl k", k=128)
            prod4 = prod_hf[:].rearrange("p (b rl k) -> p b rl k", rl=RL, k=128)
            # batch-0 multiply on DVE, batch-1 multiply concurrently on the
            # otherwise-idle gpsimd engine; both reduces stay on DVE
            _first_mul = []
            _mul_eng = [nc.vector, nc.gpsimd]
            for b in range(B):
                _first_mul.append(_mul_eng[b].tensor_mul(
                    prod4[:, b : b + 1, :, :],
                    pw3.unsqueeze(1),
                    gview[:, b : b + 1, :].unsqueeze(2).broadcast_to(
                        [128, 1, RL, 128]
                    ),
                ))
            for b in range(B):
                nc.vector.tensor_reduce(
                    red_f[:, b * RL : (b + 1) * RL],
                    prod4[:, b : b + 1, :, :],
                    mybir.AxisListType.X,
                    ALU.add,
                )
            nc.vector.tensor_copy(intT_hf[:, 0 : B * RL], red_f[:])
            # keep the DVE stream in [phi evictions, then integral] order —
            # the scheduler's cost model underestimates the AllReduce wait
            # and would otherwise queue the (blocked) muls first
            for _m in _first_mul:
                for _pe in _phi_evicts:
                    add_dep_helper(
                        _m.ins,
                        _pe.ins,
                        sync=False,
                        reason="integral DVE ops after phi evictions",
                    )

            intT2_ps = wrkhp.tile([B * RL + B, 512], HF, tag="wh", name="intT2_ps")
            nc.tensor.transpose(intT2_ps[:, 0:128], intT_hf[:], id_hf[:])
            nc.vector.tensor_copy(intRow_hf[:], intT2_ps[:, 0:128])
            nc.sync.dma_start(i_bounce[:], intRow_hf[:])
            nc.gpsimd.collective_compute(
                "AllGather",
                ALU.bypass,
                replica_groups=[list(range(NCORES))],
                ins=[i_bounce.opt()],
                outs=[i_gath.opt()],
            )

            # ---- post-AllGather: full integral, fused tail ----
            # global mean/N = sum over cores of the gathered local means;
            # the all-ones [8, 64] stationary both sums over cores and
            # broadcasts the result across 64 partitions in one matmul.
            # Then integ += psi_b * mean (the psi_b bias term).
            nc.sync.dma_start(mg_hf[:], i_gath[:, B * RL : B * RL + B, :])
            idma = [nc.gpsimd.dma_start, nc.scalar.dma_start]
            for b in range(B):
                idma[b % 2](
                    integ_hf[:R, b * C : (b + 1) * C],
                    i_gath[:, b * RL : (b + 1) * RL, :],
                )
            mg_ps = wrkp.tile([R, 512], FP, tag="w", name="mg_ps")
            nc.tensor.matmul(
                mg_ps[:, 0 : B * C], ones8_hf[:], mg_hf[:], start=True, stop=True
            )
            with nc.allow_low_precision(reason="fp16 integral bias"):
                for b in range(B):
                    nc.vector.tensor_mul(
                        mtmp_f[:],
                        psibF_hf[:],
                        mg_ps[:, b * C : (b + 1) * C],
                    )
                    nc.vector.tensor_add(
                        integ_hf[:R, b * C : (b + 1) * C],
                        integ_hf[:R, b * C : (b + 1) * C],
                        mtmp_f[:],
                    )

            # per 4-tile group: kernel_out accumulates onto the pre-computed
            # w_x PSUM bank; gelu reads PSUM directly; one 256KB store per
            # group.
            for g in range(4):
                b, h = divmod(g, 2)
                ko_ps = ko_banks[g]
                for i in range(4):
                    t = g * 4 + i
                    nc.tensor.matmul(
                        ko_ps[:, i * 128 : (i + 1) * 128],
                        phiT_hf[:, t * 128 : (t + 1) * 128],
                        integ_hf[:, b * C : (b + 1) * C],
                        start=False,
                        stop=(i == 3),
                    )
                og = outp.tile([128, 512], FP, tag="og", name=f"og{g}")
                nc.scalar.activation(og[:], ko_ps[:], AF.Gelu)
                dst = out_ext[b, h * 512 : (h + 1) * 512, :].rearrange(
                    "(t p) c -> p t c", p=128
                )
                odma = [nc.sync.dma_start, nc.gpsimd.dma_start][g % 2]
                odma(dst, og[:].rearrange("p (t c) -> p t c", c=128))

    nc.compile()
    return nc


def make_in_maps(inputs):
    x = np.asarray(inputs["x"], dtype=np.float32).astype(np.float16)
    W_w = np.asarray(inputs["W_w"], dtype=np.float32).astype(np.float16)
    W_b = (
        np.asarray(inputs["W_b"], dtype=np.float32)
        .reshape(1, C)
        .astype(np.float16)
    )
    phi_w = np.asarray(inputs["phi_w"], dtype=np.float32).astype(np.float16)
    phibT = np.ascontiguousarray(
        np.asarray(inputs["phi_b"], dtype=np.float32).reshape(R, 1)
    )
    psi_w = np.asarray(inputs["psi_w"], dtype=np.float32).astype(np.float16)
    psi_b = np.asarray(inputs["psi_b"], dtype=np.float32).astype(np.float16)
    psibF = np.ascontiguousarray(psi_b.reshape(R, C))
    ident = np.eye(128, dtype=np.float16)

    in_maps = []
    for i in range(NCORES):
        xs = x[:, i * NSH : (i + 1) * NSH, :]          # [B, NSH, C]
        xs_r = xs.reshape(B, NT, 128, C)
        xt = np.ones((128, B * NT, 129), np.float16)
        xt[:, :, :C] = xs_r.transpose(2, 0, 1, 3).reshape(128, B * NT, C)
        xT = xs.transpose(2, 0, 1).reshape(C, B * NSH)
        pw = psi_w[:, i * RL * C : (i + 1) * RL * C]
        psiwT = (
            pw.reshape(C, RL, C).transpose(2, 1, 0).reshape(C, RL * C)
        )

        in_maps.append(
            {
                "xt": np.ascontiguousarray(xt.reshape(128, B * NT * 129)),
                "xT": np.ascontiguousarray(xT),
                "psiwT": np.ascontiguousarray(psiwT),
                "psibF": psibF,
                "phi_w": phi_w,
                "phibT": phibT,
                "W_w": W_w,
                "W_b": W_b,
                "ident": ident,
            }
        )

    return in_maps


def kernel(**inputs):
    global LAST_RESULTS
    if "nc" not in _CACHE:
        _CACHE["nc"] = _build()
    nc = _CACHE["nc"]
    in_maps = make_in_maps(inputs)
    res = run_bass_kernel_spmd(nc, in_maps, core_ids=list(range(NCORES)))
    LAST_RESULTS = res
    outs = [res.results[i]["out"] for i in range(NCORES)]
    return np.concatenate(outs, axis=1)


# revision 30
# speedup vs baseline: 1.1609x; 1.1609x over previous
"""Distributed LNO block kernel for 8 TRN2 NeuronCores.

Math (reference):
    phi   = x @ phi_w + phi_b                      [B,N,r]
    psi   = (x @ psi_w + psi_b).reshape(B,N,r,C)   [B,N,r,C]  (never materialized)
    integral_c = einsum('bnrc,bnc->brc', psi, x)/N [B,r,C]
    out   = gelu(x @ W_w + W_b + phi @ integral_c)

Key identity: integral_c[b,r,c] = sum_k psi_w[k, r*C+c] * G[b,k,c] + psi_b[r*C+c] * m[b,c]
with G = (x^T x)/N (per-batch Gram, [B,C,C]) and m = mean_n x.  This removes the
34 GFLOP psi matmul entirely.

Distribution: shard N by 8 (each core: 1024 pts of both batches).
  1. per-core scaled Gram G_loc/N (+mean col via a baked-in ones column) on PE,
     AllReduce (fp16, 66KB) -> global G/N
  2. per-core r-slice (8 ranks) of the integral via one DVE broadcast-mul and
     one DVE free-axis reduce (uses G's symmetry: G^T = G, so no G transpose),
     AllGather (fp16, 4KB/core) -> full integral everywhere
  3. per-core: w_x and phi @ integral accumulate into the SAME PSUM bank (one
     accumulation group per 2KB zero-region), gelu straight out of PSUM,
     batched 256KB stores.

Everything is fp16 on the wire and in the matmuls (fp32 accumulation); inputs
are pre-cast and pre-transposed on the host so every DMA is contiguous and no
x/psi_w transposes run on the device.  rel-err vs the fp32 reference ~5e-4.
"""

import sys

sys.path.insert(0, "/opt/trn_rl_repo")

import numpy as np

import concourse.bass as bass
import concourse.bacc as bacc
import concourse.mybir as mybir
import concourse.tile as tile
from concourse.tile import add_dep_helper
from concourse.bass_utils import run_bass_kernel_spmd

FP = mybir.dt.float32
HF = mybir.dt.float16
AF = mybir.ActivationFunctionType
ALU = mybir.AluOpType

B, N, C, R = 2, 8192, 128, 64
NCORES = 8
NSH = N // NCORES      # 1024 points per core
RL = R // NCORES       # 8 rank rows per core
NT = NSH // 128        # 8 n-tiles of 128 per batch
CP1 = C + 1            # G columns + mean column

_CACHE = {}
LAST_RESULTS = None


def _build():
    nc = bacc.Bacc(
        "TRN2", target_bir_lowering=False, debug=False, num_devices=NCORES
    )

    xt_in = nc.dram_tensor("xt", [128, B * NT * 129], HF, kind="ExternalInput")
    xT_in = nc.dram_tensor("xT", [128, B * NSH], HF, kind="ExternalInput")
    psiwT_in = nc.dram_tensor("psiwT", [128, RL * C], HF, kind="ExternalInput")
    psibF_in = nc.dram_tensor("psibF", [R, C], HF, kind="ExternalInput")
    phiw_in = nc.dram_tensor("phi_w", [C, R], HF, kind="ExternalInput")
    phibT_in = nc.dram_tensor("phibT", [R, 1], FP, kind="ExternalInput")
    ww_in = nc.dram_tensor("W_w", [C, C], HF, kind="ExternalInput")
    wb_in = nc.dram_tensor("W_b", [1, C], HF, kind="ExternalInput")
    id_in = nc.dram_tensor("ident", [128, 128], HF, kind="ExternalInput")
    out_ext = nc.dram_tensor("out", [B, NSH, C], FP, kind="ExternalOutput")

    with tile.TileContext(nc) as tc:
        with (
            tc.tile_pool(name="big", bufs=1) as bigp,
            tc.tile_pool(name="outs", bufs=4) as outp,
            tc.tile_pool(name="dram", bufs=1, space="DRAM") as dramp,
            tc.tile_pool(name="gmps", bufs=1, space="PSUM") as gmp,
            tc.tile_pool(name="wrk", bufs=2, space="PSUM") as wrkp,
            tc.tile_pool(name="wrkh", bufs=1, space="PSUM") as wrkhp,
            tc.tile_pool(name="kops", bufs=4, space="PSUM") as kop,
        ):
            # ---- static SBUF tiles ----
            id_hf = bigp.tile([128, 128], HF, name="id_hf")
            x_hf = bigp.tile([128, B * NT * 129], HF, name="x_hf")
            xT_hf = bigp.tile([128, B * NSH], HF, name="xT_hf")
            psiwT_hf = bigp.tile([128, RL * C], HF, name="psiwT_hf")
            psibF_hf = bigp.tile([R, C], HF, name="psibF_hf")
            ones8_hf = bigp.tile([RL, R], HF, name="ones8_hf")
            mg_hf = bigp.tile([RL, B * C], HF, name="mg_hf")
            mtmp_f = bigp.tile([R, C], FP, name="mtmp_f")
            phiw_hf = bigp.tile([128, R], HF, name="phiw_hf")
            phibT_f = bigp.tile([R, 1], FP, name="phibT_f")
            ww_hf = bigp.tile([128, C], HF, name="ww_hf")
            phiT_hf = bigp.tile([R + 1, B * NSH], HF, name="phiT_hf")
            gloc_hf = bigp.tile([128, B * C], HF, name="gloc_hf")
            graw_hf = bigp.tile([128, B * C], HF, name="graw_hf")
            prod_hf = bigp.tile([128, B * RL * C], HF, name="prod_hf")
            red_f = bigp.tile([128, B * RL], FP, name="red_f")
            intT_hf = bigp.tile([128, B * RL + B], HF, name="intT_hf")
            intRow_hf = bigp.tile([B * RL + B, C], HF, name="intRow_hf")
            integ_hf = bigp.tile([R + 1, B * C], HF, name="integ_hf")

            # ---- DRAM bounce buffers for collectives ----
            g_bounce = dramp.tile([128, B * C], HF, name="g_bounce")
            g_red = dramp.tile(
                [128, B * C], HF, name="g_red", addr_space="Shared"
            )
            i_bounce = dramp.tile([B * RL + B, C], HF, name="i_bounce")
            i_gath = dramp.tile(
                [NCORES, B * RL + B, C], HF, name="i_gath", addr_space="Shared"
            )

            nc.vector.memset(phiT_hf[R : R + 1, :], 1.0)
            nc.vector.memset(ones8_hf[:], 1.0)
            # tiny dummy gelu: forces the ACT function tables to load now
            # instead of lazily right before the first real gelu in the tail
            warm_f = bigp.tile([1, 8], FP, name="warm_f")
            nc.vector.memset(warm_f[:], 0.0)
            nc.scalar.activation(warm_f[:], warm_f[:], AF.Gelu)

            # ---- x tiles: 6 contiguous 86KB DMAs round-robin on 3 queues ----
            q = B * NT * 129 // 6
            xq = [nc.sync.dma_start, nc.scalar.dma_start, nc.gpsimd.dma_start]
            for i in range(6):
                xq[i % 3](
                    x_hf[:, i * q : (i + 1) * q], xt_in[:, i * q : (i + 1) * q]
                )

            # ---- Gram matmuls (+ mean col via the baked-in ones column) ----
            g_ps = gmp.tile([128, B * CP1], FP, name="g_ps")
            for b in range(B):
                for j in range(NT):
                    t = b * NT + j
                    xt = x_hf[:, t * 129 : t * 129 + 128]
                    xt1 = x_hf[:, t * 129 : t * 129 + 129]
                    nc.tensor.matmul(
                        g_ps[:, b * CP1 : (b + 1) * CP1],
                        xt,
                        xt1,
                        start=(j == 0),
                        stop=(j == NT - 1),
                    )
            # evict pre-scaled by 1/N so the AllReduce carries G/N directly;
            # only the G block goes on the wire (65536B -> Mesh algorithm),
            # the local mean columns ride the AllGather instead.
            gpview = g_ps[:].rearrange("p (b w) -> p b w", w=CP1)
            bdma = [nc.scalar.dma_start, nc.sync.dma_start]
            for b in range(B):
                nc.vector.tensor_scalar_mul(
                    gloc_hf[:, b * C : (b + 1) * C],
                    gpview[:, b, 0:C],
                    1.0 / N,
                )
                bdma[b](
                    g_bounce[:, b * C : (b + 1) * C],
                    gloc_hf[:, b * C : (b + 1) * C],
                )
            nc.vector.tensor_scalar_mul(
                intT_hf[:, B * RL : B * RL + B], gpview[:, :, C], 1.0 / N
            )
            ar_cc = nc.gpsimd.collective_compute(
                "AllReduce",
                ALU.add,
                replica_groups=[list(range(NCORES))],
                ins=[g_bounce.opt()],
                outs=[g_red.opt()],
            )

            # ---- remaining loads: demoted priority so the scheduler keeps
            # them behind the critical pre-trigger chain; all of these are
            # consumed only under/after the collective waits ----
            _late = []
            _late.append(nc.scalar.dma_start(xT_hf[:], xT_in[:]))
            _late.append(nc.gpsimd.dma_start(phiw_hf[:], phiw_in[:]))
            _late.append(nc.gpsimd.dma_start(ww_hf[:], ww_in[:]))
            _late.append(nc.gpsimd.dma_start(id_hf[:], id_in[:]))
            _late.append(nc.gpsimd.dma_start(psiwT_hf[:], psiwT_in[:]))
            _late.append(nc.gpsimd.dma_start(psibF_hf[:], psibF_in[:]))
            _late.append(nc.gpsimd.dma_start(phibT_f[:], phibT_in[:]))
            for b in range(B):
                _late.append(
                    nc.gpsimd.dma_start(
                        integ_hf[R : R + 1, b * C : (b + 1) * C], wb_in[:]
                    )
                )
            for _di in _late:
                add_dep_helper(
                    _di.ins,
                    ar_cc.ins,
                    sync=False,
                    reason="load after the AllReduce trigger",
                )

            # ---- overlaps the AllReduce: phi + the w_x half of the tail ----
            _phi_evicts = []
            for ch in range(4):
                phi_ps = wrkp.tile([R, 512], FP, tag="w", name=f"phi_ps{ch}")
                cols = slice(ch * 512, (ch + 1) * 512)
                nc.tensor.matmul(
                    phi_ps[:], phiw_hf[:], xT_hf[:, cols], start=True, stop=True
                )
                _phi_evicts.append(
                    nc.vector.tensor_scalar_add(
                        phiT_hf[:R, cols], phi_ps[:], phibT_f[:]
                    )
                )

            # w_x matmuls need only xT/W_w — issue them now so they run under
            # the collective waits; phi@integral accumulates on top after the
            # AllGather lands.  One accumulation group per PSUM bank: start
            # only on the bank's first matmul (start=True clears has_written
            # for the entire 2KB zero-region).
            ko_banks = []
            for g in range(4):
                ko_ps = kop.tile([128, 512], FP, tag="ko", name=f"ko{g}")
                ko_banks.append(ko_ps)
                for i in range(4):
                    t = g * 4 + i
                    nc.tensor.matmul(
                        ko_ps[:, i * 128 : (i + 1) * 128],
                        xT_hf[:, t * 128 : (t + 1) * 128],
                        ww_hf[:],
                        start=(i == 0),
                        stop=False,
                    )

            # ---- post-AllReduce: integral r-slice on DVE, pipelined per
            # batch (each batch's 32KB G half loads on its own queue) ----
            gdma = [nc.sync.dma_start, nc.scalar.dma_start]
            for b in range(B):
                gdma[b](
                    graw_hf[:, b * C : (b + 1) * C],
                    g_red[:, b * C : (b + 1) * C],
                )
            gview = graw_hf[:].rearrange("p (b w) -> p b w", w=C)
            pw3 = psiwT_hf[:].rearrange("p (rl k) -> p rl k", k=128)
            prod4 = prod_hf[:].rearrange("p (b rl k) -> p b rl k", rl=RL, k=128)
            _first_mul = []
            for b in range(B):
                _first_mul.append(nc.vector.tensor_mul(
                    prod4[:, b : b + 1, :, :],
                    pw3.unsqueeze(1),
                    gview[:, b : b + 1, :].unsqueeze(2).broadcast_to(
                        [128, 1, RL, 128]
                    ),
                ))
                nc.vector.tensor_reduce(
                    red_f[:, b * RL : (b + 1) * RL],
                    prod4[:, b : b + 1, :, :],
                    mybir.AxisListType.X,
                    ALU.add,
                )
            nc.vector.tensor_copy(intT_hf[:, 0 : B * RL], red_f[:])
            # keep the DVE stream in [phi evictions, then integral] order —
            # the scheduler's cost model underestimates the AllReduce wait
            # and would otherwise queue the (blocked) muls first
            for _m in _first_mul:
                for _pe in _phi_evicts:
                    add_dep_helper(
                        _m.ins,
                        _pe.ins,
                        sync=False,
                        reason="integral DVE ops after phi evictions",
                    )

            intT2_ps = wrkhp.tile([B * RL + B, 512], HF, tag="wh", name="intT2_ps")
            nc.tensor.transpose(intT2_ps[:, 0:128], intT_hf[:], id_hf[:])
            nc.vector.tensor_copy(intRow_hf[:], intT2_ps[:, 0:128])
            nc.sync.dma_start(i_bounce[:], intRow_hf[:])
            nc.gpsimd.collective_compute(
                "AllGather",
                ALU.bypass,
                replica_groups=[list(range(NCORES))],
                ins=[i_bounce.opt()],
                outs=[i_gath.opt()],
            )

            # ---- post-AllGather: full integral, fused tail ----
            # global mean/N = sum over cores of the gathered local means;
            # the all-ones [8, 64] stationary both sums over cores and
            # broadcasts the result across 64 partitions in one matmul.
            # Then integ += psi_b * mean (the psi_b bias term).
            nc.sync.dma_start(mg_hf[:], i_gath[:, B * RL : B * RL + B, :])
            idma = [nc.gpsimd.dma_start, nc.scalar.dma_start]
            for b in range(B):
                idma[b % 2](
                    integ_hf[:R, b * C : (b + 1) * C],
                    i_gath[:, b * RL : (b + 1) * RL, :],
                )
            mg_ps = wrkp.tile([R, 512], FP, tag="w", name="mg_ps")
            nc.tensor.matmul(
                mg_ps[:, 0 : B * C], ones8_hf[:], mg_hf[:], start=True, stop=True
            )
            with nc.allow_low_precision(reason="fp16 integral bias"):
                for b in range(B):
                    nc.vector.tensor_mul(
                        mtmp_f[:],
                        psibF_hf[:],
                        mg_ps[:, b * C : (b + 1) * C],
                    )
                    nc.vector.tensor_add(
                        integ_hf[:R, b * C : (b + 1) * C],
                        integ_hf[:R, b * C : (b + 1) * C],
                        mtmp_f[:],
                    )

            # per 4-tile group: kernel_out accumulates onto the pre-computed
            # w_x PSUM bank; gelu reads PSUM directly; one 256KB store per
            # group.
            for g in range(4):
                b, h = divmod(g, 2)
                ko_ps = ko_banks[g]
                for i in range(4):
                    t = g * 4 + i
                    nc.tensor.matmul(
                        ko_ps[:, i * 128 : (i + 1) * 128],
                        phiT_hf[:, t * 128 : (t + 1) * 128],
                        integ_hf[:, b * C : (b + 1) * C],
                        start=False,
                        stop=(i == 3),
                    )
                og = outp.tile([128, 512], FP, tag="og", name=f"og{g}")
                nc.scalar.activation(og[:], ko_ps[:], AF.Gelu)
                dst = out_ext[b, h * 512 : (h + 1) * 512, :].rearrange(
                    "(t p) c -> p t c", p=128
                )
                odma = [nc.sync.dma_start, nc.gpsimd.dma_start][g % 2]
                odma(dst, og[:].rearrange("p (t c) -> p t c", c=128))

    nc.compile()
    return nc


def make_in_maps(inputs):
    x = np.asarray(inputs["x"], dtype=np.float32).astype(np.float16)
    W_w = np.asarray(inputs["W_w"], dtype=np.float32).astype(np.float16)
    W_b = (
        np.asarray(inputs["W_b"], dtype=np.float32)
        .reshape(1, C)
        .astype(np.float16)
    )
    phi_w = np.asarray(inputs["phi_w"], dtype=np.float32).astype(np.float16)
    phibT = np.ascontiguousarray(
        np.asarray(inputs["phi_b"], dtype=np.float32).reshape(R, 1)
    )
    psi_w = np.asarray(inputs["psi_w"], dtype=np.float32).astype(np.float16)
    psi_b = np.asarray(inputs["psi_b"], dtype=np.float32).astype(np.float16)
    psibF = np.ascontiguousarray(psi_b.reshape(R, C))
    ident = np.eye(128, dtype=np.float16)

    in_maps = []
    for i in range(NCORES):
        xs = x[:, i * NSH : (i + 1) * NSH, :]          # [B, NSH, C]
        xs_r = xs.reshape(B, NT, 128, C)
        xt = np.ones((128, B * NT, 129), np.float16)
        xt[:, :, :C] = xs_r.transpose(2, 0, 1, 3).reshape(128, B * NT, C)
        xT = xs.transpose(2, 0, 1).reshape(C, B * NSH)
        pw = psi_w[:, i * RL * C : (i + 1) * RL * C]
        psiwT = (
            pw.reshape(C, RL, C).transpose(2, 1, 0).reshape(C, RL * C)
        )

        in_maps.append(
            {
                "xt": np.ascontiguousarray(xt.reshape(128, B * NT * 129)),
                "xT": np.ascontiguousarray(xT),
                "psiwT": np.ascontiguousarray(psiwT),
                "psibF": psibF,
                "phi_w": phi_w,
                "phibT": phibT,
                "W_w": W_w,
                "W_b": W_b,
                "ident": ident,
            }
        )

    return in_maps


def kernel(**inputs):
    global LAST_RESULTS
    if "nc" not in _CACHE:
        _CACHE["nc"] = _build()
    nc = _CACHE["nc"]
    in_maps = make_in_maps(inputs)
    res = run_bass_kernel_spmd(nc, in_maps, core_ids=list(range(NCORES)))
    LAST_RESULTS = res
    outs = [res.results[i]["out"] for i in range(NCORES)]
    return np.concatenate(outs, axis=1)
